# revision 53
# baseline (speedup 1.0000x reference)
"""AttentionAvg kernel for 8 Trainium2 NeuronCores — v2.

Reference (per batch b):
    q = x @ Wq^T + bq; k = x @ Wk^T + bk          (t, d)
    s = q @ k^T / sqrt(d);  s[:, j] = -1e9 where mask[j] == 0
    w = softmax(s, axis=-1);  out[b] = sum_t x[t] * w[t, t]

Only the softmax *diagonal* is needed.  Expanding the scores,
    s[q, k] = x_q^T A x_k + u[q] + v[k] + c,
    A = Wq^T Wk / sqrt(d),   v = Wk^T bq / sqrt(d),
and the row-constant terms u[q] + c cancel in
    w[t, t] = exp(s_tt) / sum_k exp(s_tk).
So ONE projection  Y = X A  with v folded in as the per-partition
activation bias (Y_q = A^T x_q + v  =>  Y_q . x_k = x_q^T A x_k + v[k])
replaces the two d x d Q/K projections of the naive pipeline.

Masked rows/keys are dropped by a HOST-side gather of the unmasked rows,
zero-padded to a multiple of 128 (Tg ~ T/2 for this mask, shrinking the
dominant (t, t, d) matmul ~4x).  Zero-padded COLUMNS contribute exactly
exp(0) = 1 to every row sum — corrected by subtracting n_pad from Z on
device.  Zero-padded ROWS get harmless finite weights that multiply
all-zero x rows in the final matvec.  The host also pre-transposes and
casts everything, so the device performs no gathers and no transposes;
XT arrives chunked by plain DMA and the PE can start almost immediately.

Per-core device pipeline (Tile framework):
  1. per k-chunk: YT[:, :, chunk] = A^T @ XT(chunk) + v     (PE + ACT)
  2. wavefront S(ib, jc) = YT(ib)^T @ XT(jc) in PSUM; ACT exp -> SBUF
     with accum_out row-sums into Zbig; the diagonal block is extracted
     with a fused tensor_tensor_reduce against an identity mask.
  3. w = diag * 1/(Z - n_pad); out += w^T @ X via accumulating PE matvec
     against host-gathered x rows (bf16), PSUM -> SBUF -> DRAM.

Sharding: data-parallel over batch, one batch row per core (8 == 8).
"""

import math
import sys

import numpy as np

for _p in ("/opt/trn_rl_repo",):
    if _p not in sys.path:
        sys.path.insert(0, _p)

import ml_dtypes  # noqa: E402

import concourse.bass as bass  # noqa: E402
from concourse import bacc  # noqa: E402
import concourse.mybir as mybir  # noqa: E402
import concourse.tile as tile  # noqa: E402

B, T, D = 8, 4096, 768
P = 128
DT = D // P  # 6 contraction tiles
CH = 512  # free-dim chunk width (one PSUM bank of fp32)
HD = D // 2  # finalize matvec split (<=512 per PSUM bank)
NCORES = 8
SCALE = 1.0 / math.sqrt(D)

F32 = mybir.dt.float32
BF16 = mybir.dt.bfloat16
FP8 = mybir.dt.float8e4
BF = ml_dtypes.bfloat16
YSC = 16.0  # Y is scaled into fp8's normal range; exp() scale undoes it


def _chunks(n, width):
    """Remainder-FIRST chunking with a ramp: small chunks lead so the
    wavefront can start on little data (DMA head latency), full-width
    chunks follow.  The second full chunk is split in half to smooth the
    early supply/demand race."""
    out = []
    c0 = 0
    rem = n % width
    if rem:
        out.append((0, rem))
        c0 = rem
    while c0 < n:
        out.append((c0, width))
        c0 += width
    if len(out) >= 3 and out[1][1] == width:
        c0, w = out[1]
        out[1:2] = [(c0, w // 2), (c0 + w // 2, w - w // 2)]
    return out


def build_graph(nc, Tg):
    """Emit the per-core graph for gathered/padded length Tg (multiple of P)."""
    JB = Tg // P
    chunks = _chunks(Tg, CH)

    IC = len(chunks)

    # chunk-major packing: one DMA descriptor per chunk (descriptor issue
    # rate, ~0.65us/queue, limits the early supply otherwise)
    xt = nc.declare_dram_parameter("xt", [P, DT * Tg], BF16, isOutput=False)
    xt8 = nc.declare_dram_parameter("xt8", [P, DT * Tg], FP8, isOutput=False)
    aw = nc.declare_dram_parameter("aw", [P, DT * D], BF16, isOutput=False)
    vb = nc.declare_dram_parameter("vb", [P, DT], F32, isOutput=False)
    idf = nc.declare_dram_parameter("idf", [P, P], BF16, isOutput=False)
    npz = nc.declare_dram_parameter("npz", [P, JB], F32, isOutput=False)
    xg = nc.declare_dram_parameter("xg", [JB, P, D], BF16, isOutput=False)
    out = nc.declare_dram_parameter("out", [1, D], F32, isOutput=True)

    with tile.TileContext(nc) as tc:
        with (
            tc.tile_pool(name="singles", bufs=1) as singles,
            tc.tile_pool(name="spool", bufs=4) as spool,
            tc.tile_pool(name="stats", bufs=6) as stats,
            tc.tile_pool(name="psS", bufs=6, space="PSUM") as psS,
            tc.tile_pool(name="psO", bufs=1, space="PSUM") as psO,
        ):
            # ---- resident tensors ----
            XT = singles.tile([P, DT, Tg], BF16, tag="XT")
            YT = singles.tile([P, DT, Tg], BF16, tag="YT")
            # fp8 copies feed the DoubleRow score matmuls (non-diag chunks)
            XT8 = singles.tile([P, DT, Tg], FP8, tag="XT8")
            YT8 = singles.tile([P, DT, Tg], FP8, tag="YT8")
            XG = singles.tile([P, JB, D], BF16, tag="XG")
            AW = singles.tile([P, DT, D], BF16, tag="AW")
            VB = singles.tile([P, DT], F32, tag="VB")
            identity = singles.tile([P, P], BF16, tag="ident")
            # [jc, ib] layout; slot IC holds -n_pad so the plain row-sum
            # over slots comes out already pad-corrected; slots IC+1/IC+2
            # hold the fp8 flank partial sums of split diag chunks
            NZS = IC + 3
            Zbig = singles.tile([P, NZS, JB], F32, tag="Zbig")
            nc.vector.memset(Zbig[:, IC + 1 :, :], 0.0)
            diag_cols = singles.tile([P, JB], F32, tag="diag_cols")

            # ---- DMA issue: A first (needed by every Y matmul), then XT
            # chunks round-robin over queues, small singles, then xg rows
            # (only needed at finalize). ----
            # keep the Scalar queue free of DMA descriptors: it must drain
            # exp/bias activations at full rate or PSUM banks back up
            qs = (nc.sync, nc.gpsimd)
            for i in range(DT // 2):
                qs[i % 2].dma_start(
                    AW[:, 2 * i : 2 * i + 2, :],
                    aw[:, 2 * i * D : (2 * i + 2) * D],
                )
            for i, (c0, w) in enumerate(chunks):
                qs[i % 2].dma_start(
                    XT[:, :, c0 : c0 + w], xt[:, c0 * DT : (c0 + w) * DT]
                )
                qs[(i + 1) % 2].dma_start(
                    XT8[:, :, c0 : c0 + w], xt8[:, c0 * DT : (c0 + w) * DT]
                )
                if i == 0:
                    nc.sync.dma_start(VB, vb[:, :])
                elif i == 1:
                    nc.gpsimd.dma_start(Zbig[:, IC, :], npz[:, :])
                    nc.gpsimd.dma_start(identity, idf[:, :])
            # xg rows are needed only at finalize: keep them off the
            # critical queues and gate them behind mid-wavefront YT
            # progress so they don't steal HBM bandwidth from XT at the
            # head
            xg_gate = singles.tile([P, 1], BF16, tag="xg_gate")

            def emit_xg_dmas(gate_c0):
                nc.gpsimd.tensor_copy(out=xg_gate, in_=YT[:, 0, gate_c0 : gate_c0 + 1])
                for ib in range(JB):
                    nc.gpsimd.dma_start(XG[:, ib, :], xg[ib, :, :])

            po1 = psO.tile([1, HD], F32, tag="po1")
            po2 = psO.tile([1, HD], F32, tag="po2")

            # ---- PE warmup: DMA-independent dummy matmuls fill the DMA
            # head latency and push the HAM clock-gate to 8/8 before the
            # real wavefront arrives ----
            dmy = singles.tile([P, CH], BF16, tag="dmy")
            nc.vector.memset(dmy, 0.0)
            for _ in range(38):
                psw = psS.tile([P, CH], F32, tag="psS")
                nc.tensor.matmul(
                    psw[:, :256], lhsT=dmy[:, :P], rhs=dmy[:, :256],
                    start=True, stop=True,
                )

            def emit_y(c0, w):
                for eo in range(DT):
                    ps = psS.tile([P, CH], F32, tag="psS")
                    for ei in range(DT):
                        nc.tensor.matmul(
                            ps[:, :w],
                            lhsT=AW[:, ei, eo * P : (eo + 1) * P],
                            rhs=XT[:, ei, c0 : c0 + w],
                            start=(ei == 0),
                            stop=(ei == DT - 1),
                        )
                    nc.scalar.activation(
                        out=YT[:, eo, c0 : c0 + w],
                        in_=ps[:, :w],
                        func=mybir.ActivationFunctionType.Identity,
                        bias=VB[:, eo : eo + 1],
                        scale=1.0,
                    )
                    nc.vector.tensor_scalar_mul(
                        YT8[:, eo, c0 : c0 + w], YT[:, eo, c0 : c0 + w], YSC
                    )

            def mm_fp8(ps, ib, k0, kw, psoff):
                for r in range(DT // 2):
                    nc.tensor.matmul(
                        ps[:, psoff : psoff + kw],
                        lhsT=YT8[:, 2 * r : 2 * r + 2, ib * P : (ib + 1) * P],
                        rhs=XT8[:, 2 * r : 2 * r + 2, k0 : k0 + kw],
                        start=(r == 0),
                        stop=(r == DT // 2 - 1),
                        perf_mode=mybir.MatmulPerfMode.DoubleRow,
                    )

            def emit_s(ib, jc):
                c0, w = chunks[jc]
                dj = ib * P
                has_diag = c0 <= dj < c0 + w
                ps = psS.tile([P, CH], F32, tag="psS")
                e_sb = spool.tile([P, CH], BF16, tag="esb")
                split = has_diag and w > 3 * P  # flanks big enough to pay off
                if has_diag:
                    off = dj - c0
                    dw = P if split else w
                    doff = off if split else 0
                    dk = dj if split else c0
                    for et in range(DT):
                        nc.tensor.matmul(
                            ps[:, doff : doff + dw],
                            lhsT=YT[:, et, ib * P : (ib + 1) * P],
                            rhs=XT[:, et, dk : dk + dw],
                            start=(et == 0),
                            stop=(et == DT - 1),
                        )
                    if split:
                        # fp8 flanks around the bf16 diag block
                        for fi, (f0, fw) in enumerate(
                            ((0, off), (off + P, w - off - P))
                        ):
                            if fw <= 0:
                                continue
                            mm_fp8(ps, ib, c0 + f0, fw, f0)
                            nc.scalar.activation(
                                out=e_sb[:, f0 : f0 + fw],
                                in_=ps[:, f0 : f0 + fw],
                                func=mybir.ActivationFunctionType.Exp,
                                scale=1.0 / YSC,
                            )
                            nc.vector.reduce_sum(
                                Zbig[:, IC + 1 + fi, ib : ib + 1],
                                e_sb[:, f0 : f0 + fw],
                                axis=mybir.AxisListType.X,
                            )
                    nc.scalar.activation(
                        out=e_sb[:, doff : doff + dw],
                        in_=ps[:, doff : doff + dw],
                        func=mybir.ActivationFunctionType.Exp,
                    )
                    nc.vector.reduce_sum(
                        Zbig[:, jc, ib : ib + 1],
                        e_sb[:, doff : doff + dw],
                        axis=mybir.AxisListType.X,
                    )
                    dsc = spool.tile([P, P], F32, tag="dsc")
                    nc.vector.tensor_mul(dsc, e_sb[:, off : off + P], identity)
                    nc.vector.reduce_sum(
                        diag_cols[:, ib : ib + 1], dsc, axis=mybir.AxisListType.X
                    )
                else:
                    mm_fp8(ps, ib, c0, w, 0)
                    nc.scalar.activation(
                        out=e_sb[:, :w],
                        in_=ps[:, :w],
                        func=mybir.ActivationFunctionType.Exp,
                        scale=1.0 / YSC,
                    )
                    nc.vector.reduce_sum(
                        Zbig[:, jc, ib : ib + 1],
                        e_sb[:, :w],
                        axis=mybir.AxisListType.X,
                    )

            fin_n = [0]

            def emit_finalize(ib):
                z = stats.tile([P, 1], F32, tag="z")
                nc.vector.reduce_sum(
                    z, Zbig[:, :, ib : ib + 1], axis=mybir.AxisListType.XY
                )
                rz = stats.tile([P, 1], F32, tag="rz")
                nc.vector.reciprocal(rz, z)
                wcol = stats.tile([P, 1], BF16, tag="wcol")
                nc.vector.tensor_mul(wcol, diag_cols[:, ib : ib + 1], rz)
                for po, sl in ((po1, slice(0, HD)), (po2, slice(HD, D))):
                    nc.tensor.matmul(
                        po,
                        lhsT=wcol,
                        rhs=XG[:, ib, sl],
                        start=(fin_n[0] == 0),
                        stop=(fin_n[0] == JB - 1),
                    )
                fin_n[0] += 1

            # ---- wavefront: per chunk s compute YT(s), then all S(ib, jc)
            # with max(block(ib), jc) == s ----
            last = len(chunks) - 1
            for s, (c0, w) in enumerate(chunks):
                sb0 = c0 // P
                sb1 = (c0 + w + P - 1) // P
                emit_y(c0, w)
                if s == min(3, last):
                    emit_xg_dmas(c0)
                # older rows vs the fresh chunk right after Y's matmuls:
                # they only need the chunk DMA + old YT8, so they fill the
                # PE while the new rows' Y ACT -> fp8-cast chain drains
                for ib in range(0, sb0):
                    emit_s(ib, s)
                    if s == last:
                        emit_finalize(ib)
                for ib in range(sb0, sb1):
                    for jc in range(s + 1):
                        emit_s(ib, jc)
                    if s == last:
                        emit_finalize(ib)

            out_sb = singles.tile([1, D], F32, tag="out_sb")
            nc.vector.tensor_copy(out=out_sb[:, :HD], in_=po1)
            nc.scalar.activation(
                out=out_sb[:, HD:],
                in_=po2,
                func=mybir.ActivationFunctionType.Copy,
            )
            nc.sync.dma_start(out[:, :], out_sb)

    return nc


def prepare_host_inputs(inputs, mask):
    """Per-batch gather + zero-pad to the common padded length Tg."""
    idxs, counts = [], []
    for b in range(B):
        nz = np.nonzero(mask[b])[0]
        idxs.append(nz)
        counts.append(len(nz))
    Tg = max(max(counts), 1)
    Tg = ((Tg + P - 1) // P) * P
    return Tg, idxs, counts


def build_in_maps(inputs, mask, Wq_w, Wq_b, Wk_w, Wk_b, Tg, idxs, counts):
    JB = Tg // P
    # s * Wq^T Wk  and  s * Wk^T bq  (row-constant score terms cancel)
    A = (np.asarray(Wq_w, np.float32).T @ np.asarray(Wk_w, np.float32)) * SCALE
    vvec = (np.asarray(Wk_w, np.float32).T @ np.asarray(Wq_b, np.float32)) * SCALE
    aw_arr = np.ascontiguousarray(
        A.astype(BF).reshape(DT, P, D).transpose(1, 0, 2).reshape(P, DT * D)
    )
    vb_arr = np.ascontiguousarray(vvec.reshape(DT, P).T)
    idf = np.eye(P, dtype=np.float32).astype(BF)
    chunks = _chunks(Tg, CH)

    def chunk_major(xtT):
        """[DT, P, Tg] -> [P, sum(DT*w)] with per-chunk [DT, w] blocks."""
        return np.concatenate(
            [
                xtT[:, :, c0 : c0 + w].transpose(1, 0, 2).reshape(P, DT * w)
                for c0, w in chunks
            ],
            axis=1,
        )

    in_maps = []
    for b in range(B):
        n = counts[b]
        xg_f = np.zeros((Tg, D), np.float32)
        if n:
            xg_f[:n] = inputs[b][idxs[b]]
        xg_bf = xg_f.astype(BF)
        xtT = xg_bf.T.reshape(DT, P, Tg)
        xt_arr = chunk_major(xtT)
        xt8_arr = chunk_major(xtT.astype(ml_dtypes.float8_e4m3fn))
        xg_arr = xg_bf.reshape(JB, P, D)
        in_maps.append(
            {
                "xt": xt_arr,
                "xt8": xt8_arr,
                "aw": aw_arr,
                "vb": vb_arr,
                "idf": idf,
                "npz": np.full((P, JB), -float(Tg - n), np.float32),
                "xg": xg_arr,
            }
        )
    return in_maps


def kernel(inputs, mask, Wq_w, Wq_b, Wk_w, Wk_b, qk_bf16=True, _trace=False):
    from concourse.bass_utils import run_bass_kernel_spmd

    inputs = np.ascontiguousarray(inputs, np.float32)
    mask = np.asarray(mask)
    Tg, idxs, counts = prepare_host_inputs(inputs, mask)

    nc = bacc.Bacc()
    build_graph(nc, Tg)
    nc.compile()

    in_maps = build_in_maps(inputs, mask, Wq_w, Wq_b, Wk_w, Wk_b, Tg, idxs, counts)

    res = run_bass_kernel_spmd(
        nc, in_maps, core_ids=list(range(NCORES)), trace=_trace
    )
    out = np.stack([res.results[b]["out"][0] for b in range(B)], axis=0)

    # degenerate all-masked batch: softmax over a constant row is uniform
    for b in range(B):
        if counts[b] == 0:
            out[b] = inputs[b].mean(axis=0)

    if _trace:
        return out, res
    return out


# revision 56
# speedup vs baseline: 1.0347x; 1.0347x over previous
"""AttentionAvg kernel for 8 Trainium2 NeuronCores — v2.

Reference (per batch b):
    q = x @ Wq^T + bq; k = x @ Wk^T + bk          (t, d)
    s = q @ k^T / sqrt(d);  s[:, j] = -1e9 where mask[j] == 0
    w = softmax(s, axis=-1);  out[b] = sum_t x[t] * w[t, t]

Only the softmax *diagonal* is needed.  Expanding the scores,
    s[q, k] = x_q^T A x_k + u[q] + v[k] + c,
    A = Wq^T Wk / sqrt(d),   v = Wk^T bq / sqrt(d),
and the row-constant terms u[q] + c cancel in
    w[t, t] = exp(s_tt) / sum_k exp(s_tk).
So ONE projection  Y = X A  with v folded in as the per-partition
activation bias (Y_q = A^T x_q + v  =>  Y_q . x_k = x_q^T A x_k + v[k])
replaces the two d x d Q/K projections of the naive pipeline.

Masked rows/keys are dropped by a HOST-side gather of the unmasked rows,
zero-padded to a multiple of 128 (Tg ~ T/2 for this mask, shrinking the
dominant (t, t, d) matmul ~4x).  Zero-padded COLUMNS contribute exactly
exp(0) = 1 to every row sum — corrected by subtracting n_pad from Z on
device.  Zero-padded ROWS get harmless finite weights that multiply
all-zero x rows in the final matvec.  The host also pre-transposes and
casts everything, so the device performs no gathers and no transposes;
XT arrives chunked by plain DMA and the PE can start almost immediately.

Per-core device pipeline (Tile framework):
  1. per k-chunk: YT[:, :, chunk] = A^T @ XT(chunk) + v     (PE + ACT)
  2. wavefront S(ib, jc) = YT(ib)^T @ XT(jc) in PSUM; ACT exp -> SBUF
     with accum_out row-sums into Zbig; the diagonal block is extracted
     with a fused tensor_tensor_reduce against an identity mask.
  3. w = diag * 1/(Z - n_pad); out += w^T @ X via accumulating PE matvec
     against host-gathered x rows (bf16), PSUM -> SBUF -> DRAM.

Sharding: data-parallel over batch, one batch row per core (8 == 8).
"""

import math
import sys

import numpy as np

for _p in ("/opt/trn_rl_repo",):
    if _p not in sys.path:
        sys.path.insert(0, _p)

import ml_dtypes  # noqa: E402

import concourse.bass as bass  # noqa: E402
from concourse import bacc  # noqa: E402
import concourse.mybir as mybir  # noqa: E402
import concourse.tile as tile  # noqa: E402

B, T, D = 8, 4096, 768
P = 128
DT = D // P  # 6 contraction tiles
CH = 512  # free-dim chunk width (one PSUM bank of fp32)
HD = D // 2  # finalize matvec split (<=512 per PSUM bank)
NCORES = 8
SCALE = 1.0 / math.sqrt(D)

F32 = mybir.dt.float32
BF16 = mybir.dt.bfloat16
FP8 = mybir.dt.float8e4
BF = ml_dtypes.bfloat16
YSC = 16.0  # Y is scaled into fp8's normal range; exp() scale undoes it


def _chunks(n, width):
    """Remainder-FIRST chunking with a ramp: small chunks lead so the
    wavefront can start on little data (DMA head latency), full-width
    chunks follow.  The second full chunk is split in half to smooth the
    early supply/demand race."""
    out = []
    c0 = 0
    rem = n % width
    if rem:
        out.append((0, rem))
        c0 = rem
    while c0 < n:
        out.append((c0, width))
        c0 += width
    if len(out) >= 3 and out[1][1] == width:
        c0, w = out[1]
        out[1:2] = [(c0, w // 2), (c0 + w // 2, w - w // 2)]
    return out


def build_graph(nc, Tg):
    """Emit the per-core graph for gathered/padded length Tg (multiple of P)."""
    JB = Tg // P
    chunks = _chunks(Tg, CH)

    IC = len(chunks)

    # chunk-major packing: one DMA descriptor per chunk (descriptor issue
    # rate, ~0.65us/queue, limits the early supply otherwise)
    xt = nc.declare_dram_parameter("xt", [P, DT * Tg], BF16, isOutput=False)
    xt8 = nc.declare_dram_parameter("xt8", [P, DT * Tg], FP8, isOutput=False)
    aw = nc.declare_dram_parameter("aw", [P, DT * D], BF16, isOutput=False)
    vb = nc.declare_dram_parameter("vb", [P, DT], F32, isOutput=False)
    idf = nc.declare_dram_parameter("idf", [P, P], BF16, isOutput=False)
    npz = nc.declare_dram_parameter("npz", [P, JB], F32, isOutput=False)
    xg = nc.declare_dram_parameter("xg", [JB, P, D], BF16, isOutput=False)
    out = nc.declare_dram_parameter("out", [1, D], F32, isOutput=True)

    with tile.TileContext(nc) as tc:
        with (
            tc.tile_pool(name="singles", bufs=1) as singles,
            tc.tile_pool(name="spool", bufs=4) as spool,
            tc.tile_pool(name="stats", bufs=6) as stats,
            tc.tile_pool(name="psS", bufs=6, space="PSUM") as psS,
            tc.tile_pool(name="psO", bufs=1, space="PSUM") as psO,
        ):
            # ---- resident tensors ----
            XT = singles.tile([P, DT, Tg], BF16, tag="XT")
            YT = singles.tile([P, DT, Tg], BF16, tag="YT")
            # fp8 copies feed the DoubleRow score matmuls (non-diag chunks)
            XT8 = singles.tile([P, DT, Tg], FP8, tag="XT8")
            YT8 = singles.tile([P, DT, Tg], FP8, tag="YT8")
            XG = singles.tile([P, JB, D], BF16, tag="XG")
            AW = singles.tile([P, DT, D], BF16, tag="AW")
            VB = singles.tile([P, DT], F32, tag="VB")
            identity = singles.tile([P, P], BF16, tag="ident")
            # [jc, ib] layout; slot IC holds -n_pad so the plain row-sum
            # over slots comes out already pad-corrected; slots IC+1/IC+2
            # hold the fp8 flank partial sums of split diag chunks
            NZS = IC + 3
            Zbig = singles.tile([P, NZS, JB], F32, tag="Zbig")
            nc.vector.memset(Zbig[:, IC + 1 :, :], 0.0)
            diag_cols = singles.tile([P, JB], F32, tag="diag_cols")

            # ---- DMA issue: A first (needed by every Y matmul), then XT
            # chunks round-robin over queues, small singles, then xg rows
            # (only needed at finalize). ----
            # keep the Scalar queue free of DMA descriptors: it must drain
            # exp/bias activations at full rate or PSUM banks back up
            # three queues for the head-critical transfers (the scalar
            # queue is idle until its first exp ~14us, so a few early
            # descriptors there are free); later chunks avoid scalar
            q3 = (nc.sync, nc.gpsimd, nc.scalar)
            for i in range(DT // 2):
                q3[i % 3].dma_start(
                    AW[:, 2 * i : 2 * i + 2, :],
                    aw[:, 2 * i * D : (2 * i + 2) * D],
                )
            qs = (nc.sync, nc.gpsimd)
            for i, (c0, w) in enumerate(chunks):
                qa = q3 if i < 2 else qs
                qa[i % len(qa)].dma_start(
                    XT[:, :, c0 : c0 + w], xt[:, c0 * DT : (c0 + w) * DT]
                )
                qa[(i + 1) % len(qa)].dma_start(
                    XT8[:, :, c0 : c0 + w], xt8[:, c0 * DT : (c0 + w) * DT]
                )
                if i == 0:
                    nc.scalar.dma_start(VB, vb[:, :])
                elif i == 1:
                    nc.gpsimd.dma_start(Zbig[:, IC, :], npz[:, :])
                    nc.gpsimd.dma_start(identity, idf[:, :])
            # xg rows are needed only at finalize: keep them off the
            # critical queues and gate them behind mid-wavefront YT
            # progress so they don't steal HBM bandwidth from XT at the
            # head
            xg_gate = singles.tile([P, 1], BF16, tag="xg_gate")

            def emit_xg_dmas(gate_c0):
                nc.gpsimd.tensor_copy(out=xg_gate, in_=YT[:, 0, gate_c0 : gate_c0 + 1])
                for ib in range(JB):
                    nc.gpsimd.dma_start(XG[:, ib, :], xg[ib, :, :])

            po1 = psO.tile([1, HD], F32, tag="po1")
            po2 = psO.tile([1, HD], F32, tag="po2")

            # ---- PE warmup: DMA-independent dummy matmuls fill the DMA
            # head latency and push the HAM clock-gate to 8/8 before the
            # real wavefront arrives ----
            dmy = singles.tile([P, CH], BF16, tag="dmy")
            nc.vector.memset(dmy, 0.0)
            for _ in range(28):
                psw = psS.tile([P, CH], F32, tag="psS")
                nc.tensor.matmul(
                    psw[:, :256], lhsT=dmy[:, :P], rhs=dmy[:, :256],
                    start=True, stop=True,
                )

            def emit_y(c0, w):
                for eo in range(DT):
                    ps = psS.tile([P, CH], F32, tag="psS")
                    for ei in range(DT):
                        nc.tensor.matmul(
                            ps[:, :w],
                            lhsT=AW[:, ei, eo * P : (eo + 1) * P],
                            rhs=XT[:, ei, c0 : c0 + w],
                            start=(ei == 0),
                            stop=(ei == DT - 1),
                        )
                    nc.scalar.activation(
                        out=YT[:, eo, c0 : c0 + w],
                        in_=ps[:, :w],
                        func=mybir.ActivationFunctionType.Identity,
                        bias=VB[:, eo : eo + 1],
                        scale=1.0,
                    )
                    nc.vector.tensor_scalar_mul(
                        YT8[:, eo, c0 : c0 + w], YT[:, eo, c0 : c0 + w], YSC
                    )

            def mm_fp8(ps, ib, k0, kw, psoff):
                for r in range(DT // 2):
                    nc.tensor.matmul(
                        ps[:, psoff : psoff + kw],
                        lhsT=YT8[:, 2 * r : 2 * r + 2, ib * P : (ib + 1) * P],
                        rhs=XT8[:, 2 * r : 2 * r + 2, k0 : k0 + kw],
                        start=(r == 0),
                        stop=(r == DT // 2 - 1),
                        perf_mode=mybir.MatmulPerfMode.DoubleRow,
                    )

            def emit_s(ib, jc):
                c0, w = chunks[jc]
                dj = ib * P
                has_diag = c0 <= dj < c0 + w
                ps = psS.tile([P, CH], F32, tag="psS")
                e_sb = spool.tile([P, CH], BF16, tag="esb")
                split = has_diag and w > 3 * P  # flanks big enough to pay off
                if has_diag:
                    off = dj - c0
                    dw = P if split else w
                    doff = off if split else 0
                    dk = dj if split else c0
                    for et in range(DT):
                        nc.tensor.matmul(
                            ps[:, doff : doff + dw],
                            lhsT=YT[:, et, ib * P : (ib + 1) * P],
                            rhs=XT[:, et, dk : dk + dw],
                            start=(et == 0),
                            stop=(et == DT - 1),
                        )
                    if split:
                        # fp8 flanks around the bf16 diag block
                        for fi, (f0, fw) in enumerate(
                            ((0, off), (off + P, w - off - P))
                        ):
                            if fw <= 0:
                                continue
                            mm_fp8(ps, ib, c0 + f0, fw, f0)
                            nc.scalar.activation(
                                out=e_sb[:, f0 : f0 + fw],
                                in_=ps[:, f0 : f0 + fw],
                                func=mybir.ActivationFunctionType.Exp,
                                scale=1.0 / YSC,
                            )
                            nc.vector.reduce_sum(
                                Zbig[:, IC + 1 + fi, ib : ib + 1],
                                e_sb[:, f0 : f0 + fw],
                                axis=mybir.AxisListType.X,
                            )
                    nc.scalar.activation(
                        out=e_sb[:, doff : doff + dw],
                        in_=ps[:, doff : doff + dw],
                        func=mybir.ActivationFunctionType.Exp,
                    )
                    nc.vector.reduce_sum(
                        Zbig[:, jc, ib : ib + 1],
                        e_sb[:, doff : doff + dw],
                        axis=mybir.AxisListType.X,
                    )
                    dsc = spool.tile([P, P], F32, tag="dsc")
                    nc.vector.tensor_mul(dsc, e_sb[:, off : off + P], identity)
                    nc.vector.reduce_sum(
                        diag_cols[:, ib : ib + 1], dsc, axis=mybir.AxisListType.X
                    )
                else:
                    mm_fp8(ps, ib, c0, w, 0)
                    nc.scalar.activation(
                        out=e_sb[:, :w],
                        in_=ps[:, :w],
                        func=mybir.ActivationFunctionType.Exp,
                        scale=1.0 / YSC,
                    )
                    nc.vector.reduce_sum(
                        Zbig[:, jc, ib : ib + 1],
                        e_sb[:, :w],
                        axis=mybir.AxisListType.X,
                    )

            fin_n = [0]

            def emit_finalize(ib):
                z = stats.tile([P, 1], F32, tag="z")
                nc.vector.reduce_sum(
                    z, Zbig[:, :, ib : ib + 1], axis=mybir.AxisListType.XY
                )
                rz = stats.tile([P, 1], F32, tag="rz")
                nc.vector.reciprocal(rz, z)
                wcol = stats.tile([P, 1], BF16, tag="wcol")
                nc.vector.tensor_mul(wcol, diag_cols[:, ib : ib + 1], rz)
                for po, sl in ((po1, slice(0, HD)), (po2, slice(HD, D))):
                    nc.tensor.matmul(
                        po,
                        lhsT=wcol,
                        rhs=XG[:, ib, sl],
                        start=(fin_n[0] == 0),
                        stop=(fin_n[0] == JB - 1),
                    )
                fin_n[0] += 1

            # ---- wavefront: per chunk s compute YT(s), then all S(ib, jc)
            # with max(block(ib), jc) == s ----
            last = len(chunks) - 1
            for s, (c0, w) in enumerate(chunks):
                sb0 = c0 // P
                sb1 = (c0 + w + P - 1) // P
                emit_y(c0, w)
                if s == min(3, last):
                    emit_xg_dmas(c0)
                for ib in range(sb0, sb1):
                    for jc in range(s + 1):
                        emit_s(ib, jc)
                    if s == last:
                        emit_finalize(ib)
                for ib in range(0, sb0):
                    emit_s(ib, s)
                    if s == last:
                        emit_finalize(ib)

            out_sb = singles.tile([1, D], F32, tag="out_sb")
            nc.vector.tensor_copy(out=out_sb[:, :HD], in_=po1)
            nc.scalar.activation(
                out=out_sb[:, HD:],
                in_=po2,
                func=mybir.ActivationFunctionType.Copy,
            )
            nc.sync.dma_start(out[:, :], out_sb)

    return nc


def prepare_host_inputs(inputs, mask):
    """Per-batch gather + zero-pad to the common padded length Tg."""
    idxs, counts = [], []
    for b in range(B):
        nz = np.nonzero(mask[b])[0]
        idxs.append(nz)
        counts.append(len(nz))
    Tg = max(max(counts), 1)
    Tg = ((Tg + P - 1) // P) * P
    return Tg, idxs, counts


def build_in_maps(inputs, mask, Wq_w, Wq_b, Wk_w, Wk_b, Tg, idxs, counts):
    JB = Tg // P
    # s * Wq^T Wk  and  s * Wk^T bq  (row-constant score terms cancel)
    A = (np.asarray(Wq_w, np.float32).T @ np.asarray(Wk_w, np.float32)) * SCALE
    vvec = (np.asarray(Wk_w, np.float32).T @ np.asarray(Wq_b, np.float32)) * SCALE
    aw_arr = np.ascontiguousarray(
        A.astype(BF).reshape(DT, P, D).transpose(1, 0, 2).reshape(P, DT * D)
    )
    vb_arr = np.ascontiguousarray(vvec.reshape(DT, P).T)
    idf = np.eye(P, dtype=np.float32).astype(BF)
    chunks = _chunks(Tg, CH)

    def chunk_major(xtT):
        """[DT, P, Tg] -> [P, sum(DT*w)] with per-chunk [DT, w] blocks."""
        return np.concatenate(
            [
                xtT[:, :, c0 : c0 + w].transpose(1, 0, 2).reshape(P, DT * w)
                for c0, w in chunks
            ],
            axis=1,
        )

    in_maps = []
    for b in range(B):
        n = counts[b]
        xg_f = np.zeros((Tg, D), np.float32)
        if n:
            xg_f[:n] = inputs[b][idxs[b]]
        xg_bf = xg_f.astype(BF)
        xtT = xg_bf.T.reshape(DT, P, Tg)
        xt_arr = chunk_major(xtT)
        xt8_arr = chunk_major(xtT.astype(ml_dtypes.float8_e4m3fn))
        xg_arr = xg_bf.reshape(JB, P, D)
        in_maps.append(
            {
                "xt": xt_arr,
                "xt8": xt8_arr,
                "aw": aw_arr,
                "vb": vb_arr,
                "idf": idf,
                "npz": np.full((P, JB), -float(Tg - n), np.float32),
                "xg": xg_arr,
            }
        )
    return in_maps


def kernel(inputs, mask, Wq_w, Wq_b, Wk_w, Wk_b, qk_bf16=True, _trace=False):
    from concourse.bass_utils import run_bass_kernel_spmd

    inputs = np.ascontiguousarray(inputs, np.float32)
    mask = np.asarray(mask)
    Tg, idxs, counts = prepare_host_inputs(inputs, mask)

    nc = bacc.Bacc()
    build_graph(nc, Tg)
    nc.compile()

    in_maps = build_in_maps(inputs, mask, Wq_w, Wq_b, Wk_w, Wk_b, Tg, idxs, counts)

    res = run_bass_kernel_spmd(
        nc, in_maps, core_ids=list(range(NCORES)), trace=_trace
    )
    out = np.stack([res.results[b]["out"][0] for b in range(B)], axis=0)

    # degenerate all-masked batch: softmax over a constant row is uniform
    for b in range(B):
        if counts[b] == 0:
            out[b] = inputs[b].mean(axis=0)

    if _trace:
        return out, res
    return out
